# revision 17
# baseline (speedup 1.0000x reference)
"""PASA group-softmax downsample kernel for 8 Trainium2 NeuronCores.

Reference computation (per reference.py):
  x (2, 64, 32, 32, 32) f32
  xp = reflect-pad x by 1 on d/h/w
  sigma = conv3d(xp, conv_w (54, 64, 3,3,3), stride 1, valid)   -> (2, 54, 32,32,32)
  sigma = batchnorm(sigma, batch stats over (n,d,h,w), gamma, beta)
  sigma = softmax(sigma, axis=1)                                 -> (2, 54, ...)
  out[n,g,cc,o] = sum_p patches[n,g,cc,p,o] * sigma[n,g*27+p,o]  (g=2 groups of 32 ch)
  return out[:, :, ::2, ::2, ::2]                                -> (2, 64, 16, 16, 16)

Sharding: 8 shards = (batch n in {0,1}) x (4 depth chunks of 8 planes).
Each core gets a padded depth slab with halo (10 planes of the padded
volume), computes its local conv at full stride-1 resolution (needed for
the global BN statistics), AllReduces the per-channel sum/sumsq (54x2
floats), then computes softmax + the adaptive weighted sum only at the
strided output positions it owns, and writes (64, 4, 16, 16).

Conv strategy: 27 taps as shifted matmuls accumulated in PSUM,
contraction over input channels (K=64). The input slab is stored twice
in SBUF ([128, .] tensor, upper half shifted by one element along w) so
taps (wl=0, wl=1) fuse into one K=128 matmul; wl=2 taps run at K=64.
18 matmuls per 512-position group, dtype float32r (full PE rate, N>=256).

Post stage (per 128-position tile, positions on partitions):
  E = exp(a*sigma_sub + b) (one ACT op, channel-major), PE-transpose to
  position-major, PE-transpose the 27 patch views, one DVE multiply with
  a free-dim-broadcast attention view, one fused reduce over taps, scale
  by 1/Z (per-partition scalar), PE-transpose back, DMA out.
"""

import sys

sys.path.insert(0, "/opt/trn_rl_repo")

import numpy as np

import concourse.bacc as bacc
import concourse.mybir as mybir
from concourse import bass_utils, tile

N_CORES = 8
K = 3
GROUP = 2
STRIDE = 2
EPS = 1e-5

N, C, D, H, W = 2, 64, 32, 32, 32
COUT = GROUP * K * K * K  # 54
PD, PH, PW = D + 2, H + 2, W + 2  # 34, 34, 34
ZPLANES = 10  # 8 output planes + 2 halo planes of the padded volume
PLANE = PH * PW  # 1156
XLEN = ZPLANES * PLANE  # 11560
XBUF = XLEN + 8  # pad: junk-column reads of the last plane overrun slightly
DL = 8  # local output depth extent (stride-1)
POS = DL * H * W  # 8192 stride-1 positions per core
SPOS = (DL // 2) * (H // 2) * (W // 2)  # 1024 strided positions per core
NTILES = SPOS // 128  # 8 position tiles
M_TOTAL = float(N * D * H * W)  # 65536 positions for BN stats

F32 = mybir.dt.float32
F32R = mybir.dt.float32r

# tap index p = di*9 + hj*3 + wl (matches reference im2col ordering)
# conv matmul units: 9 pairs (wl=0 with wl=1 via the shifted copy) + 9 singles
PAIR_UNITS = [(di, hj, 0) for di in range(K) for hj in range(K)]
SINGLE_UNITS = [(di, hj, 2) for di in range(K) for hj in range(K)]
UNITS = [(di, hj, wl, 128) for (di, hj, wl) in PAIR_UNITS] + [
    (di, hj, wl, 64) for (di, hj, wl) in SINGLE_UNITS
]
NUNITS = len(UNITS)  # 18

_PROGRAM_CACHE = {}


def _build_weight_pack(conv_w: np.ndarray) -> np.ndarray:
    """Pack conv_w (54, 64, 3, 3, 3) into lhsT layout (128, 18*54)."""
    wpk = np.zeros((128, NUNITS * COUT), dtype=np.float32)
    for u, (di, hj, wl, ku) in enumerate(UNITS):
        # lhsT[k, m]: k = input channel (row), m = output channel
        wpk[0:64, u * COUT : (u + 1) * COUT] = conv_w[:, :, di, hj, wl].T
        if ku == 128:
            wpk[64:128, u * COUT : (u + 1) * COUT] = conv_w[:, :, di, hj, wl + 1].T
    return wpk


def _build_program():
    nc = bacc.Bacc(
        "TRN2", target_bir_lowering=False, debug=False, num_devices=N_CORES
    )
    xp2 = nc.dram_tensor("xp2", (128, XBUF), F32, kind="ExternalInput").ap()
    wpk = nc.dram_tensor("wpk", (128, NUNITS * COUT), F32, kind="ExternalInput").ap()
    gb = nc.dram_tensor("gb", (COUT, 2), F32, kind="ExternalInput").ap()
    out = nc.dram_tensor("out", (64, SPOS), F32, kind="ExternalOutput").ap()

    with tile.TileContext(nc) as tc:
        _emit(nc, tc, xp2, wpk, gb, out)
    nc.compile()
    return nc


def _win(t, parts, offset, dims):
    """Overlapping-window AP view of a [P, L] tile: free dims [(step, count), ...]."""
    v = t[0:parts, offset : offset + 1]
    for _ in range(len(dims) - 1):
        v = v.unsqueeze(1)
    w = v.copy()
    for i, (st, cnt) in enumerate(dims):
        w.ap[i + 1] = (st, cnt)
    return w


def _emit(nc, tc, xp2, wpk, gb, out):
    AX = mybir.AxisListType
    OP = mybir.AluOpType
    ACT = mybir.ActivationFunctionType

    with (
        tc.tile_pool(name="xin", bufs=1) as xin_pool,
        tc.tile_pool(name="consts", bufs=1) as const_pool,
        tc.tile_pool(name="stats", bufs=1) as stats_pool,
        tc.tile_pool(name="sq", bufs=2) as sq_pool,
        tc.tile_pool(name="post", bufs=1) as post_pool,
        tc.tile_pool(name="dram", bufs=1, space="DRAM") as dram_pool,
    ):
        XPR = xin_pool.tile([128, XBUF], F32R)
        WPK = const_pool.tile([128, NUNITS * COUT], F32)
        WPKR = const_pool.tile([128, NUNITS * COUT], F32R)
        GB = const_pool.tile([COUT, 2], F32)

        # input DMAs: stage per-plane, round f32 -> f32r into the slab
        for z in range(ZPLANES):
            lo = z * PLANE
            hi = XBUF if z == ZPLANES - 1 else (z + 1) * PLANE
            STG = sq_pool.tile([128, PLANE + 8], F32, tag="stg")
            nc.sync.dma_start(STG[:, 0 : hi - lo], xp2[:, lo:hi])
            nc.vector.tensor_copy(XPR[:, lo:hi], STG[:, 0 : hi - lo])
        nc.sync.dma_start(WPK[:], wpk[:])
        nc.vector.tensor_copy(WPKR[:], WPK[:])
        nc.sync.dma_start(GB[:], gb[:])

        XP4 = XPR[:, 0:XLEN].rearrange("c (z y x) -> c z y x", z=ZPLANES, y=PH, x=PW)

        SUMS = stats_pool.tile([COUT, DL], F32)
        SUMSQ = stats_pool.tile([COUT, DL], F32)
        SSUB = stats_pool.tile([COUT, SPOS], F32)  # strided sigma, channel-major

        # ---- conv + stats + strided extraction ----
        # Matmul rhs must be a single contiguous free dim, so each depth
        # plane is computed over the flat span h*34+w for h<32, w<34 (1088
        # positions incl. 2 junk columns per row) in 3 matmul groups.
        SPAN = (PH - 2) * PW  # 1088
        GRP = [(0, 364), (364, 364), (728, 360)]
        with tc.tile_pool(name="psum_conv", bufs=4, space="PSUM") as pconv:
            for d in range(DL):
                SIG = sq_pool.tile([COUT, SPAN], F32, tag="sig")
                for j0, ns in GRP:
                    P = pconv.tile([COUT, 384], F32, tag="convps")
                    for u, (di, hj, wl, ku) in enumerate(UNITS):
                        lhsT = WPKR[0:ku, u * COUT : (u + 1) * COUT]
                        rhs = XPR[
                            0:ku,
                            (d + di) * PLANE + hj * PW + wl + j0 :
                            (d + di) * PLANE + hj * PW + wl + j0 + ns,
                        ]
                        nc.tensor.matmul(
                            P[:, 0:ns], lhsT, rhs,
                            start=(u == 0), stop=(u == NUNITS - 1),
                        )
                    nc.scalar.copy(SIG[:, j0 : j0 + ns], P[:, 0:ns])
                # stats over the 32x32 valid positions of this plane
                SIGv = SIG.rearrange("c (h w) -> c h w", h=H, w=PW)[:, :, 0:W]
                nc.vector.tensor_reduce(
                    SUMS[:, d : d + 1], SIGv, axis=AX.XY, op=OP.add
                )
                SQT = sq_pool.tile([COUT, H * W], F32, tag="sqt")
                SQTv = SQT.rearrange("c (h w) -> c h w", h=H, w=W)
                nc.scalar.activation(
                    SQTv, SIGv, ACT.Square, accum_out=SUMSQ[:, d : d + 1]
                )
                if d % 2 == 0:
                    nc.scalar.copy(
                        SSUB[:, (d // 2) * 256 : (d // 2) * 256 + 256],
                        SIGv[:, 0:H:2, 0:W:2],
                    )

        # ---- finalize local stats, AllReduce, compute a/b ----
        ST = stats_pool.tile([COUT, 2], F32)
        nc.vector.tensor_reduce(ST[:, 0:1], SUMS[:], axis=AX.X, op=OP.add)
        nc.vector.tensor_reduce(ST[:, 1:2], SUMSQ[:], axis=AX.X, op=OP.add)

        cc_in = dram_pool.tile([COUT, 2], F32)
        cc_out = dram_pool.tile([COUT, 2], F32)
        nc.sync.dma_start(cc_in[:], ST[:])
        nc.gpsimd.collective_compute(
            "AllReduce",
            OP.add,
            ins=[cc_in.opt()],
            outs=[cc_out.opt()],
            replica_groups=[list(range(N_CORES))],
        )
        GST = stats_pool.tile([COUT, 2], F32)
        nc.sync.dma_start(GST[:], cc_out[:])

        MEAN = stats_pool.tile([COUT, 1], F32)
        VAR = stats_pool.tile([COUT, 1], F32)
        STD = stats_pool.tile([COUT, 1], F32)
        RSTD = stats_pool.tile([COUT, 1], F32)
        A_ = stats_pool.tile([COUT, 1], F32)
        B_ = stats_pool.tile([COUT, 1], F32)
        T1 = stats_pool.tile([COUT, 1], F32)
        nc.vector.tensor_scalar_mul(MEAN[:], GST[:, 0:1], 1.0 / M_TOTAL)
        # var = E[x^2] - mean^2 = sumsq/M - mean*mean
        nc.vector.tensor_scalar_mul(VAR[:], GST[:, 1:2], 1.0 / M_TOTAL)
        nc.vector.tensor_mul(T1[:], MEAN[:], MEAN[:])
        nc.vector.tensor_sub(VAR[:], VAR[:], T1[:])
        EPST = stats_pool.tile([COUT, 1], F32)
        nc.vector.memset(EPST[:], float(EPS))
        nc.scalar.activation(STD[:], VAR[:], ACT.Sqrt, bias=EPST[:])
        nc.vector.reciprocal(RSTD[:], STD[:])
        # a = gamma * rstd ; b = beta - mean * a
        nc.vector.tensor_mul(A_[:], GB[:, 0:1], RSTD[:])
        nc.vector.tensor_mul(T1[:], MEAN[:], A_[:])
        nc.vector.tensor_sub(B_[:], GB[:, 1:2], T1[:])

        # ---- E = exp(a*sigma + b), channel-major ----
        E = post_pool.tile([COUT, SPOS], F32)
        nc.scalar.activation(E[:], SSUB[:], ACT.Exp, bias=B_[:], scale=A_[:])

        # ---- softmax denominator Z via ones-matmul colsum ----
        ONES = stats_pool.tile([COUT, 1], F32)
        nc.vector.memset(ONES[:], 1.0)
        ZROW = post_pool.tile([1, SPOS], F32)
        with tc.tile_pool(name="psum_z", bufs=2, space="PSUM") as pz:
            for k in range(2):
                PZ = pz.tile([1, 512], F32, tag="pz")
                nc.tensor.matmul(
                    PZ[:], ONES[:], E[:, k * 512 : (k + 1) * 512],
                    start=True, stop=True,
                )
                nc.vector.reciprocal(ZROW[:, k * 512 : (k + 1) * 512], PZ[:])

        # bounce E and 1/Z through DRAM for partition-broadcast replication
        eb = dram_pool.tile([COUT, SPOS], F32)
        zb = dram_pool.tile([1, SPOS], F32)
        nc.sync.dma_start(eb[:], E[:])
        nc.sync.dma_start(zb[:], ZROW[:])
        ZREP = post_pool.tile([64, SPOS], F32)
        nc.sync.dma_start(ZREP[:], zb[0:1, :].partition_broadcast(64))

        # ---- adaptive weighted sum, channel-major, 256-position chunks ----
        OUTC = post_pool.tile([64, SPOS], F32)  # channel-major result
        CH = 256  # one strided depth plane per chunk
        for k in range(4):  # chunk = strided depth plane d4 = k
            AREP = post_pool.tile([64, 27 * CH], F32, tag="arep")
            for grp in range(2):
                nc.sync.dma_start(
                    AREP[grp * 32 : (grp + 1) * 32, :].rearrange(
                        "c (p o) -> c p o", p=27, o=CH
                    ),
                    eb[
                        grp * 27 : (grp + 1) * 27, k * CH : (k + 1) * CH
                    ].partition_broadcast(32),
                )
            PRD = post_pool.tile([64, 27 * CH], F32, tag="prd")
            for di in range(K):
                for hj in range(K):
                    # patches [c, wl, h, w] = XPR[c, base + hj*34 + wl + h*68 + w*2]
                    xv = _win(
                        XPR, 64, (2 * k + di) * PLANE + hj * PW,
                        [(1, 3), (2 * PW, 16), (2, 16)],
                    )
                    sl = slice((di * 9 + hj * 3) * CH, (di * 9 + hj * 3 + 3) * CH)
                    av = AREP[:, sl].rearrange(
                        "c (wl h w) -> c wl h w", wl=3, h=16, w=16
                    )
                    pvd = PRD[:, sl].rearrange(
                        "c (wl h w) -> c wl h w", wl=3, h=16, w=16
                    )
                    nc.vector.tensor_tensor(pvd, xv, av, op=OP.mult)
            # reduce over taps (view with p innermost)
            rv = PRD.rearrange("c (p o) -> c o p", p=27, o=CH)
            nc.vector.tensor_reduce(
                OUTC[:, k * CH : (k + 1) * CH], rv, axis=AX.X, op=OP.add
            )
        nc.vector.tensor_mul(OUTC[:], OUTC[:], ZREP[:])
        nc.sync.dma_start(out[:], OUTC[:])


def _prep_inputs(x, conv_w, bn_gamma, bn_beta):
    """Build per-core input maps from the full inputs."""
    xpad = np.pad(
        np.asarray(x, dtype=np.float32),
        ((0, 0), (0, 0), (1, 1), (1, 1), (1, 1)),
        mode="reflect",
    )
    wpk = _build_weight_pack(np.asarray(conv_w, dtype=np.float32))
    gbv = np.stack(
        [np.asarray(bn_gamma, np.float32), np.asarray(bn_beta, np.float32)], axis=1
    )
    in_maps = []
    for core in range(N_CORES):
        n, dc = core // 4, core % 4
        slab = xpad[n, :, 8 * dc : 8 * dc + ZPLANES].reshape(C, XLEN)
        xp2 = np.zeros((128, XBUF), dtype=np.float32)
        xp2[0:64, :XLEN] = slab
        xp2[64:128, : XLEN - 1] = slab[:, 1:]
        in_maps.append({"xp2": xp2, "wpk": wpk, "gb": gbv})
    return in_maps


def kernel(x, conv_w, bn_gamma, bn_beta):
    if "prog" not in _PROGRAM_CACHE:
        _PROGRAM_CACHE["prog"] = _build_program()
    nc = _PROGRAM_CACHE["prog"]
    in_maps = _prep_inputs(x, conv_w, bn_gamma, bn_beta)
    res = bass_utils.run_bass_kernel_spmd(
        nc, in_maps, core_ids=list(range(N_CORES))
    )
    full = np.empty((N, C, D // 2, H // 2, W // 2), dtype=np.float32)
    for core in range(N_CORES):
        n, dc = core // 4, core % 4
        full[n, :, 4 * dc : 4 * dc + 4] = res.results[core]["out"].reshape(
            64, 4, 16, 16
        )
    return full


# revision 20
# speedup vs baseline: 1.6306x; 1.6306x over previous
"""PASA group-softmax downsample kernel for 8 Trainium2 NeuronCores.

Reference computation (per reference.py):
  x (2, 64, 32, 32, 32) f32
  xp = reflect-pad x by 1 on d/h/w
  sigma = conv3d(xp, conv_w (54, 64, 3,3,3), stride 1, valid)   -> (2, 54, 32,32,32)
  sigma = batchnorm(sigma, batch stats over (n,d,h,w), gamma, beta)
  sigma = softmax(sigma, axis=1)                                 -> (2, 54, ...)
  out[n,g,cc,o] = sum_p patches[n,g,cc,p,o] * sigma[n,g*27+p,o]  (g=2 groups of 32 ch)
  return out[:, :, ::2, ::2, ::2]                                -> (2, 64, 16, 16, 16)

Sharding: 8 shards = (batch n in {0,1}) x (4 depth chunks of 8 planes).
Each core gets a padded depth slab with halo (10 planes of the padded
volume), computes its local conv at full stride-1 resolution (needed for
the global BN statistics), AllReduces the per-channel sum/sumsq (54x2
floats), then computes softmax + the adaptive weighted sum only at the
strided output positions it owns, and writes (64, 4, 16, 16).

Conv strategy: 27 taps as shifted matmuls accumulated in PSUM,
contraction over input channels (K=64). The input slab is stored twice
in SBUF ([128, .] tensor, upper half shifted by one element along w) so
taps (wl=0, wl=1) fuse into one K=128 matmul; wl=2 taps run at K=64.
18 matmuls per 512-position group, dtype float32r (full PE rate, N>=256).

Post stage (per 128-position tile, positions on partitions):
  E = exp(a*sigma_sub + b) (one ACT op, channel-major), PE-transpose to
  position-major, PE-transpose the 27 patch views, one DVE multiply with
  a free-dim-broadcast attention view, one fused reduce over taps, scale
  by 1/Z (per-partition scalar), PE-transpose back, DMA out.
"""

import sys

sys.path.insert(0, "/opt/trn_rl_repo")

import numpy as np

import concourse.bacc as bacc
import concourse.mybir as mybir
from concourse import bass_utils, tile

N_CORES = 8
K = 3
GROUP = 2
STRIDE = 2
EPS = 1e-5

N, C, D, H, W = 2, 64, 32, 32, 32
COUT = GROUP * K * K * K  # 54
PD, PH, PW = D + 2, H + 2, W + 2  # 34, 34, 34
ZPLANES = 10  # 8 output planes + 2 halo planes of the padded volume
PLANE = PH * PW  # 1156
XLEN = ZPLANES * PLANE  # 11560
XBUF = XLEN + 8  # pad: junk-column reads of the last plane overrun slightly
DL = 8  # local output depth extent (stride-1)
POS = DL * H * W  # 8192 stride-1 positions per core
SPOS = (DL // 2) * (H // 2) * (W // 2)  # 1024 strided positions per core
NTILES = SPOS // 128  # 8 position tiles
M_TOTAL = float(N * D * H * W)  # 65536 positions for BN stats

F32 = mybir.dt.float32
F32R = mybir.dt.float32r

# tap index p = di*9 + hj*3 + wl (matches reference im2col ordering)
# conv matmul units: 9 pairs (wl=0 with wl=1 via the shifted copy) + 9 singles
PAIR_UNITS = [(di, hj, 0) for di in range(K) for hj in range(K)]
SINGLE_UNITS = [(di, hj, 2) for di in range(K) for hj in range(K)]
UNITS = [(di, hj, wl, 128) for (di, hj, wl) in PAIR_UNITS] + [
    (di, hj, wl, 64) for (di, hj, wl) in SINGLE_UNITS
]
NUNITS = len(UNITS)  # 18

_PROGRAM_CACHE = {}


def _build_weight_pack(conv_w: np.ndarray) -> np.ndarray:
    """Pack conv_w (54, 64, 3, 3, 3) into lhsT layout (128, 18*54)."""
    wpk = np.zeros((128, NUNITS * COUT), dtype=np.float32)
    for u, (di, hj, wl, ku) in enumerate(UNITS):
        # lhsT[k, m]: k = input channel (row), m = output channel
        wpk[0:64, u * COUT : (u + 1) * COUT] = conv_w[:, :, di, hj, wl].T
        if ku == 128:
            wpk[64:128, u * COUT : (u + 1) * COUT] = conv_w[:, :, di, hj, wl + 1].T
    return wpk


def _build_program():
    nc = bacc.Bacc(
        "TRN2", target_bir_lowering=False, debug=False, num_devices=N_CORES
    )
    xp2 = nc.dram_tensor("xp2", (128, XBUF), F32, kind="ExternalInput").ap()
    wpk = nc.dram_tensor("wpk", (128, NUNITS * COUT), F32, kind="ExternalInput").ap()
    gb = nc.dram_tensor("gb", (COUT, 2), F32, kind="ExternalInput").ap()
    out = nc.dram_tensor("out", (64, SPOS), F32, kind="ExternalOutput").ap()

    with tile.TileContext(nc) as tc:
        _emit(nc, tc, xp2, wpk, gb, out)
    nc.compile()
    return nc


def _win(t, parts, offset, dims):
    """Overlapping-window AP view of a [P, L] tile: free dims [(step, count), ...]."""
    v = t[0:parts, offset : offset + 1]
    for _ in range(len(dims) - 1):
        v = v.unsqueeze(1)
    w = v.copy()
    for i, (st, cnt) in enumerate(dims):
        w.ap[i + 1] = (st, cnt)
    return w


def _emit(nc, tc, xp2, wpk, gb, out):
    AX = mybir.AxisListType
    OP = mybir.AluOpType
    ACT = mybir.ActivationFunctionType

    with (
        tc.tile_pool(name="xin", bufs=1) as xin_pool,
        tc.tile_pool(name="consts", bufs=1) as const_pool,
        tc.tile_pool(name="stats", bufs=1) as stats_pool,
        tc.tile_pool(name="sq", bufs=2) as sq_pool,
        tc.tile_pool(name="post", bufs=1) as post_pool,
        tc.tile_pool(name="dram", bufs=1, space="DRAM") as dram_pool,
    ):
        XPR = xin_pool.tile([128, XBUF], F32R)
        WPK = const_pool.tile([128, NUNITS * COUT], F32)
        WPKR = const_pool.tile([128, NUNITS * COUT], F32R)
        GB = const_pool.tile([COUT, 2], F32)

        # input DMAs: stage per-plane, round f32 -> f32r into the slab
        for z in range(ZPLANES):
            lo = z * PLANE
            hi = XBUF if z == ZPLANES - 1 else (z + 1) * PLANE
            STG = sq_pool.tile([128, PLANE + 8], F32, tag="stg")
            nc.sync.dma_start(STG[:, 0 : hi - lo], xp2[:, lo:hi])
            nc.vector.tensor_copy(XPR[:, lo:hi], STG[:, 0 : hi - lo])
        nc.sync.dma_start(WPK[:], wpk[:])
        nc.vector.tensor_copy(WPKR[:], WPK[:])
        nc.sync.dma_start(GB[:], gb[:])

        XP4 = XPR[:, 0:XLEN].rearrange("c (z y x) -> c z y x", z=ZPLANES, y=PH, x=PW)

        SUMS = stats_pool.tile([COUT, DL], F32)
        SUMSQ = stats_pool.tile([COUT, DL], F32)
        SSUB = stats_pool.tile([COUT, SPOS], F32)  # strided sigma, channel-major

        # ---- conv + stats + strided extraction ----
        # Matmul rhs must be a single contiguous free dim, so each depth
        # plane is computed over the flat span h*34+w for h<32, w<34 (1088
        # positions incl. 2 junk columns per row) in 3 matmul groups.
        SPAN = (PH - 2) * PW  # 1088
        GRP = [(0, 364), (364, 364), (728, 360)]
        with tc.tile_pool(name="psum_conv", bufs=4, space="PSUM") as pconv:
            for d in range(DL):
                SIG = sq_pool.tile([COUT, SPAN], F32, tag="sig")
                for j0, ns in GRP:
                    P = pconv.tile([COUT, 384], F32, tag="convps")
                    for u, (di, hj, wl, ku) in enumerate(UNITS):
                        lhsT = WPKR[0:ku, u * COUT : (u + 1) * COUT]
                        rhs = XPR[
                            0:ku,
                            (d + di) * PLANE + hj * PW + wl + j0 :
                            (d + di) * PLANE + hj * PW + wl + j0 + ns,
                        ]
                        nc.tensor.matmul(
                            P[:, 0:ns], lhsT, rhs,
                            start=(u == 0), stop=(u == NUNITS - 1),
                        )
                    nc.scalar.copy(SIG[:, j0 : j0 + ns], P[:, 0:ns])
                # stats over the 32x32 valid positions of this plane
                SIGv = SIG.rearrange("c (h w) -> c h w", h=H, w=PW)[:, :, 0:W]
                nc.vector.tensor_reduce(
                    SUMS[:, d : d + 1], SIGv, axis=AX.XY, op=OP.add
                )
                SQT = sq_pool.tile([COUT, H * W], F32, tag="sqt")
                SQTv = SQT.rearrange("c (h w) -> c h w", h=H, w=W)
                nc.scalar.activation(
                    SQTv, SIGv, ACT.Square, accum_out=SUMSQ[:, d : d + 1]
                )
                if d % 2 == 0:
                    nc.scalar.copy(
                        SSUB[:, (d // 2) * 256 : (d // 2) * 256 + 256],
                        SIGv[:, 0:H:2, 0:W:2],
                    )

        # ---- finalize local stats, AllReduce, compute a/b ----
        ST = stats_pool.tile([COUT, 2], F32)
        nc.vector.tensor_reduce(ST[:, 0:1], SUMS[:], axis=AX.X, op=OP.add)
        nc.vector.tensor_reduce(ST[:, 1:2], SUMSQ[:], axis=AX.X, op=OP.add)

        cc_in = dram_pool.tile([COUT, 2], F32)
        cc_out = dram_pool.tile([COUT, 2], F32)
        nc.sync.dma_start(cc_in[:], ST[:])
        nc.gpsimd.collective_compute(
            "AllReduce",
            OP.add,
            ins=[cc_in.opt()],
            outs=[cc_out.opt()],
            replica_groups=[list(range(N_CORES))],
        )
        GST = stats_pool.tile([COUT, 2], F32)
        nc.sync.dma_start(GST[:], cc_out[:])

        MEAN = stats_pool.tile([COUT, 1], F32)
        VAR = stats_pool.tile([COUT, 1], F32)
        STD = stats_pool.tile([COUT, 1], F32)
        RSTD = stats_pool.tile([COUT, 1], F32)
        A_ = stats_pool.tile([COUT, 1], F32)
        B_ = stats_pool.tile([COUT, 1], F32)
        T1 = stats_pool.tile([COUT, 1], F32)
        nc.vector.tensor_scalar_mul(MEAN[:], GST[:, 0:1], 1.0 / M_TOTAL)
        # var = E[x^2] - mean^2 = sumsq/M - mean*mean
        nc.vector.tensor_scalar_mul(VAR[:], GST[:, 1:2], 1.0 / M_TOTAL)
        nc.vector.tensor_mul(T1[:], MEAN[:], MEAN[:])
        nc.vector.tensor_sub(VAR[:], VAR[:], T1[:])
        EPST = stats_pool.tile([COUT, 1], F32)
        nc.vector.memset(EPST[:], float(EPS))
        nc.scalar.activation(STD[:], VAR[:], ACT.Sqrt, bias=EPST[:])
        nc.vector.reciprocal(RSTD[:], STD[:])
        # a = gamma * rstd ; b = beta - mean * a
        nc.vector.tensor_mul(A_[:], GB[:, 0:1], RSTD[:])
        nc.vector.tensor_mul(T1[:], MEAN[:], A_[:])
        nc.vector.tensor_sub(B_[:], GB[:, 1:2], T1[:])

        # ---- E = exp(a*sigma + b), channel-major ----
        E = post_pool.tile([COUT, SPOS], F32)
        nc.scalar.activation(E[:], SSUB[:], ACT.Exp, bias=B_[:], scale=A_[:])

        # ---- softmax denominator Z via ones-matmul colsum ----
        ONES = stats_pool.tile([COUT, 1], F32)
        nc.vector.memset(ONES[:], 1.0)
        # bounce E and Z through DRAM for partition-broadcast replication
        eb = dram_pool.tile([COUT, SPOS], F32)
        zb = dram_pool.tile([1, SPOS], F32)
        nc.sync.dma_start(eb[:], E[:])
        ZROW = post_pool.tile([1, SPOS], F32)
        with tc.tile_pool(name="psum_z", bufs=2, space="PSUM") as pz:
            for k in range(2):
                PZ = pz.tile([1, 512], F32, tag="pz")
                nc.tensor.matmul(
                    PZ[:], ONES[:], E[:, k * 512 : (k + 1) * 512],
                    start=True, stop=True,
                )
                nc.scalar.copy(ZROW[:, k * 512 : (k + 1) * 512], PZ[:])
        nc.sync.dma_start(zb[:], ZROW[:])
        ZREP = post_pool.tile([64, SPOS], F32)
        ZINV = post_pool.tile([64, SPOS], F32)
        nc.sync.dma_start(ZREP[:], zb[0:1, :].partition_broadcast(64))
        nc.vector.reciprocal(ZINV[:], ZREP[:])

        # ---- adaptive weighted sum, channel-major, 256-position chunks ----
        OUTC = post_pool.tile([64, SPOS], F32)  # channel-major result
        CH = 256  # one strided depth plane per chunk
        for k in range(4):  # chunk = strided depth plane d4 = k
            AREP = post_pool.tile([64, 27 * CH], F32, tag="arep", bufs=2)
            for grp in range(2):
                nc.sync.dma_start(
                    AREP[grp * 32 : (grp + 1) * 32, :].rearrange(
                        "c (p o) -> c p o", p=27, o=CH
                    ),
                    eb[
                        grp * 27 : (grp + 1) * 27, k * CH : (k + 1) * CH
                    ].partition_broadcast(32),
                )
            PRD = post_pool.tile([64, 27 * CH], F32, tag="prd", bufs=1)
            for di in range(K):
                for hj in range(K):
                    # patches [c, wl, h, w] = XPR[c, base + hj*34 + wl + h*68 + w*2]
                    xv = _win(
                        XPR, 64, (2 * k + di) * PLANE + hj * PW,
                        [(1, 3), (2 * PW, 16), (2, 16)],
                    )
                    sl = slice((di * 9 + hj * 3) * CH, (di * 9 + hj * 3 + 3) * CH)
                    av = AREP[:, sl].rearrange(
                        "c (wl h w) -> c wl h w", wl=3, h=16, w=16
                    )
                    pvd = PRD[:, sl].rearrange(
                        "c (wl h w) -> c wl h w", wl=3, h=16, w=16
                    )
                    nc.vector.tensor_tensor(pvd, xv, av, op=OP.mult)
            # reduce over taps: contiguous halving tree on the (p, o) layout
            for lo, hi in [(11, 16), (8, 8), (4, 4), (2, 2)]:
                nc.vector.tensor_add(
                    PRD[:, 0 : lo * CH],
                    PRD[:, 0 : lo * CH],
                    PRD[:, hi * CH : (hi + lo) * CH],
                )
            nc.vector.tensor_add(
                PRD[:, 0:CH], PRD[:, 0:CH], PRD[:, CH : 2 * CH]
            )
            # fold in 1/Z while writing the chunk result
            nc.vector.tensor_mul(
                OUTC[:, k * CH : (k + 1) * CH],
                PRD[:, 0:CH],
                ZINV[:, k * CH : (k + 1) * CH],
            )
        nc.sync.dma_start(out[:], OUTC[:])


def _prep_inputs(x, conv_w, bn_gamma, bn_beta):
    """Build per-core input maps from the full inputs."""
    xpad = np.pad(
        np.asarray(x, dtype=np.float32),
        ((0, 0), (0, 0), (1, 1), (1, 1), (1, 1)),
        mode="reflect",
    )
    wpk = _build_weight_pack(np.asarray(conv_w, dtype=np.float32))
    gbv = np.stack(
        [np.asarray(bn_gamma, np.float32), np.asarray(bn_beta, np.float32)], axis=1
    )
    in_maps = []
    for core in range(N_CORES):
        n, dc = core // 4, core % 4
        slab = xpad[n, :, 8 * dc : 8 * dc + ZPLANES].reshape(C, XLEN)
        xp2 = np.zeros((128, XBUF), dtype=np.float32)
        xp2[0:64, :XLEN] = slab
        xp2[64:128, : XLEN - 1] = slab[:, 1:]
        in_maps.append({"xp2": xp2, "wpk": wpk, "gb": gbv})
    return in_maps


def kernel(x, conv_w, bn_gamma, bn_beta):
    if "prog" not in _PROGRAM_CACHE:
        _PROGRAM_CACHE["prog"] = _build_program()
    nc = _PROGRAM_CACHE["prog"]
    in_maps = _prep_inputs(x, conv_w, bn_gamma, bn_beta)
    res = bass_utils.run_bass_kernel_spmd(
        nc, in_maps, core_ids=list(range(N_CORES))
    )
    full = np.empty((N, C, D // 2, H // 2, W // 2), dtype=np.float32)
    for core in range(N_CORES):
        n, dc = core // 4, core % 4
        full[n, :, 4 * dc : 4 * dc + 4] = res.results[core]["out"].reshape(
            64, 4, 16, 16
        )
    return full


# revision 21
# speedup vs baseline: 2.0697x; 1.2693x over previous
"""PASA group-softmax downsample kernel for 8 Trainium2 NeuronCores.

Reference computation (per reference.py):
  x (2, 64, 32, 32, 32) f32
  xp = reflect-pad x by 1 on d/h/w
  sigma = conv3d(xp, conv_w (54, 64, 3,3,3), stride 1, valid)   -> (2, 54, 32,32,32)
  sigma = batchnorm(sigma, batch stats over (n,d,h,w), gamma, beta)
  sigma = softmax(sigma, axis=1)
  out[n,g,cc,o] = sum_p patches[n,g,cc,p,o] * sigma[n,g*27+p,o]  (g=2 groups of 32 ch)
  return out[:, :, ::2, ::2, ::2]                                -> (2, 64, 16, 16, 16)

Sharding: 8 shards = (batch n in {0,1}) x (4 depth chunks of 8 planes).
Each core gets a padded depth slab with halo (10 planes of the padded
volume).

Two SPMD launches (a cross-core AllReduce measured ~80us +-70us of
launch-skew in this environment, vs a ~15us bare-launch floor, so the
432-byte BN-stat reduction is done on the host between launches):

Launch A (per core): stride-1 conv of the local slab as 27 shifted
  matmuls accumulated in PSUM (wl=0/1 tap pairs fused to K=128 via a
  +1-shifted copy of the slab in partitions 64..127; float32r at
  N>=256 runs 1 cycle/row). Per-plane BN partial sums + sum-of-squares
  and the strided-position sigma are extracted on DVE/ACT. Outputs:
  st (54, 2) partial stats, ssub (54, 1024) strided conv values.

Host: sum stats over cores -> mean/var -> a, b; E = exp(a*ssub + b);
  en = E / colsum(E)  (the group-softmax attention, pre-normalized).

Launch B (per core): for each strided depth plane (256 positions),
  replicate en rows across the 32 channels of each group with a
  partition-broadcast DMA, multiply against overlapping-window views
  of the fp32 slab (9 DVE multiplies of [64, 3x16x16]), reduce the 27
  taps with a contiguous halving tree, and DMA out (64, 4, 16, 16).
"""

import sys

sys.path.insert(0, "/opt/trn_rl_repo")

import numpy as np

import concourse.bacc as bacc
import concourse.mybir as mybir
from concourse import bass_utils, tile

N_CORES = 8
K = 3
GROUP = 2
STRIDE = 2
EPS = 1e-5

N, C, D, H, W = 2, 64, 32, 32, 32
COUT = GROUP * K * K * K  # 54
PD, PH, PW = D + 2, H + 2, W + 2  # 34, 34, 34
ZPLANES = 10  # 8 output planes + 2 halo planes of the padded volume
PLANE = PH * PW  # 1156
XLEN = ZPLANES * PLANE  # 11560
XBUF = XLEN + 8  # pad: junk-column reads of the last plane overrun slightly
DL = 8  # local output depth extent (stride-1)
SPOS = (DL // 2) * (H // 2) * (W // 2)  # 1024 strided positions per core
M_TOTAL = float(N * D * H * W)  # 65536 positions for BN stats

F32 = mybir.dt.float32
F32R = mybir.dt.float32r

# tap index p = di*9 + hj*3 + wl (matches reference im2col ordering)
# conv matmul units: 9 pairs (wl=0 with wl=1 via the shifted copy) + 9 singles
PAIR_UNITS = [(di, hj, 0) for di in range(K) for hj in range(K)]
SINGLE_UNITS = [(di, hj, 2) for di in range(K) for hj in range(K)]
UNITS = [(di, hj, wl, 128) for (di, hj, wl) in PAIR_UNITS] + [
    (di, hj, wl, 64) for (di, hj, wl) in SINGLE_UNITS
]
NUNITS = len(UNITS)  # 18

_PROGRAM_CACHE = {}


def _build_weight_pack(conv_w: np.ndarray) -> np.ndarray:
    """Pack conv_w (54, 64, 3, 3, 3) into lhsT layout (128, 18*54)."""
    wpk = np.zeros((128, NUNITS * COUT), dtype=np.float32)
    for u, (di, hj, wl, ku) in enumerate(UNITS):
        # lhsT[k, m]: k = input channel (row), m = output channel
        wpk[0:64, u * COUT : (u + 1) * COUT] = conv_w[:, :, di, hj, wl].T
        if ku == 128:
            wpk[64:128, u * COUT : (u + 1) * COUT] = conv_w[:, :, di, hj, wl + 1].T
    return wpk


def _win(t, parts, offset, dims):
    """Overlapping-window AP view of a [P, L] tile: free dims [(step, count), ...]."""
    v = t[0:parts, offset : offset + 1]
    for _ in range(len(dims) - 1):
        v = v.unsqueeze(1)
    w = v.copy()
    for i, (st, cnt) in enumerate(dims):
        w.ap[i + 1] = (st, cnt)
    return w


def _build_program_a():
    nc = bacc.Bacc(
        "TRN2", target_bir_lowering=False, debug=False, num_devices=N_CORES
    )
    xp2 = nc.dram_tensor("xp2", (128, XBUF), F32, kind="ExternalInput").ap()
    wpk = nc.dram_tensor("wpk", (128, NUNITS * COUT), F32, kind="ExternalInput").ap()
    st = nc.dram_tensor("st", (COUT, 2), F32, kind="ExternalOutput").ap()
    ssub = nc.dram_tensor("ssub", (COUT, SPOS), F32, kind="ExternalOutput").ap()

    AX = mybir.AxisListType
    OP = mybir.AluOpType
    ACT = mybir.ActivationFunctionType

    with tile.TileContext(nc) as tc:
        with (
            tc.tile_pool(name="xin", bufs=1) as xin_pool,
            tc.tile_pool(name="consts", bufs=1) as const_pool,
            tc.tile_pool(name="stats", bufs=1) as stats_pool,
            tc.tile_pool(name="sq", bufs=2) as sq_pool,
        ):
            XPR = xin_pool.tile([128, XBUF], F32R)
            WPK = const_pool.tile([128, NUNITS * COUT], F32)
            WPKR = const_pool.tile([128, NUNITS * COUT], F32R)

            nc.sync.dma_start(WPK[:], wpk[:])
            nc.vector.tensor_copy(WPKR[:], WPK[:])
            # input DMAs: stage per-plane, round f32 -> f32r into the slab
            for z in range(ZPLANES):
                lo = z * PLANE
                hi = XBUF if z == ZPLANES - 1 else (z + 1) * PLANE
                STG = sq_pool.tile([128, PLANE + 8], F32, tag="stg")
                nc.sync.dma_start(STG[:, 0 : hi - lo], xp2[:, lo:hi])
                nc.vector.tensor_copy(XPR[:, lo:hi], STG[:, 0 : hi - lo])

            SUMS = stats_pool.tile([COUT, DL], F32)
            SUMSQ = stats_pool.tile([COUT, DL], F32)
            SSUB = stats_pool.tile([COUT, SPOS], F32)

            # conv: per depth plane, flat span h*34+w (h<32, w<34; 2 junk
            # cols/row discarded at the stats/extraction step) in 3 groups
            SPAN = (PH - 2) * PW  # 1088
            GRPS = [(0, 364), (364, 364), (728, 360)]
            with tc.tile_pool(name="psum_conv", bufs=4, space="PSUM") as pconv:
                for d in range(DL):
                    SIG = sq_pool.tile([COUT, SPAN], F32, tag="sig")
                    for j0, ns in GRPS:
                        P = pconv.tile([COUT, 384], F32, tag="convps")
                        for u, (di, hj, wl, ku) in enumerate(UNITS):
                            lhsT = WPKR[0:ku, u * COUT : (u + 1) * COUT]
                            base = (d + di) * PLANE + hj * PW + wl + j0
                            rhs = XPR[0:ku, base : base + ns]
                            nc.tensor.matmul(
                                P[:, 0:ns], lhsT, rhs,
                                start=(u == 0), stop=(u == NUNITS - 1),
                            )
                        nc.scalar.copy(SIG[:, j0 : j0 + ns], P[:, 0:ns])
                    # stats over the 32x32 valid positions of this plane
                    SIGv = SIG.rearrange("c (h w) -> c h w", h=H, w=PW)[:, :, 0:W]
                    nc.vector.tensor_reduce(
                        SUMS[:, d : d + 1], SIGv, axis=AX.XY, op=OP.add
                    )
                    SQT = sq_pool.tile([COUT, H * W], F32, tag="sqt")
                    SQTv = SQT.rearrange("c (h w) -> c h w", h=H, w=W)
                    nc.scalar.activation(
                        SQTv, SIGv, ACT.Square, accum_out=SUMSQ[:, d : d + 1]
                    )
                    if d % 2 == 0:
                        nc.scalar.copy(
                            SSUB[:, (d // 2) * 256 : (d // 2) * 256 + 256],
                            SIGv[:, 0:H:2, 0:W:2],
                        )

            ST = stats_pool.tile([COUT, 2], F32)
            nc.vector.tensor_reduce(ST[:, 0:1], SUMS[:], axis=AX.X, op=OP.add)
            nc.vector.tensor_reduce(ST[:, 1:2], SUMSQ[:], axis=AX.X, op=OP.add)
            nc.sync.dma_start(st[:], ST[:])
            nc.sync.dma_start(ssub[:], SSUB[:])
    nc.compile()
    return nc


def _build_program_b():
    nc = bacc.Bacc(
        "TRN2", target_bir_lowering=False, debug=False, num_devices=N_CORES
    )
    xpb = nc.dram_tensor("xpb", (64, XBUF), F32, kind="ExternalInput").ap()
    en = nc.dram_tensor("en", (COUT, SPOS), F32, kind="ExternalInput").ap()
    out = nc.dram_tensor("out", (64, SPOS), F32, kind="ExternalOutput").ap()

    OP = mybir.AluOpType
    CH = 256  # one strided depth plane per chunk

    with tile.TileContext(nc) as tc:
        with (
            tc.tile_pool(name="xin", bufs=1) as xin_pool,
            tc.tile_pool(name="work", bufs=1) as work_pool,
        ):
            XPB = xin_pool.tile([64, XBUF], F32)
            # load the slab in chunks so chunk-0 compute starts early
            bounds = [0, 3 * PLANE, 5 * PLANE, 7 * PLANE, XBUF]
            for i in range(4):
                nc.sync.dma_start(
                    XPB[:, bounds[i] : bounds[i + 1]],
                    xpb[:, bounds[i] : bounds[i + 1]],
                )
            OUTC = work_pool.tile([64, SPOS], F32)
            for k in range(4):  # chunk = strided depth plane d4 = k
                AREP = work_pool.tile([64, 27 * CH], F32, tag="arep", bufs=2)
                for grp in range(2):
                    nc.sync.dma_start(
                        AREP[grp * 32 : (grp + 1) * 32, :].rearrange(
                            "c (p o) -> c p o", p=27, o=CH
                        ),
                        en[
                            grp * 27 : (grp + 1) * 27, k * CH : (k + 1) * CH
                        ].partition_broadcast(32),
                    )
                PRD = work_pool.tile([64, 27 * CH], F32, tag="prd", bufs=1)
                for di in range(K):
                    for hj in range(K):
                        # patches [c, wl, h, w] = XPB[c, base + hj*34 + wl + h*68 + w*2]
                        xv = _win(
                            XPB, 64, (2 * k + di) * PLANE + hj * PW,
                            [(1, 3), (2 * PW, 16), (2, 16)],
                        )
                        sl = slice(
                            (di * 9 + hj * 3) * CH, (di * 9 + hj * 3 + 3) * CH
                        )
                        av = AREP[:, sl].rearrange(
                            "c (wl h w) -> c wl h w", wl=3, h=16, w=16
                        )
                        pvd = PRD[:, sl].rearrange(
                            "c (wl h w) -> c wl h w", wl=3, h=16, w=16
                        )
                        nc.vector.tensor_tensor(pvd, xv, av, op=OP.mult)
                # reduce the 27 taps: contiguous halving tree on (p, o)
                for lo, hi in [(11, 16), (8, 8), (4, 4), (2, 2)]:
                    nc.vector.tensor_add(
                        PRD[:, 0 : lo * CH],
                        PRD[:, 0 : lo * CH],
                        PRD[:, hi * CH : (hi + lo) * CH],
                    )
                nc.vector.tensor_add(
                    OUTC[:, k * CH : (k + 1) * CH],
                    PRD[:, 0:CH],
                    PRD[:, CH : 2 * CH],
                )
                nc.sync.dma_start(
                    out[:, k * CH : (k + 1) * CH], OUTC[:, k * CH : (k + 1) * CH]
                )
    nc.compile()
    return nc


def _prep_inputs_a(x, conv_w):
    xpad = np.pad(
        np.asarray(x, dtype=np.float32),
        ((0, 0), (0, 0), (1, 1), (1, 1), (1, 1)),
        mode="reflect",
    )
    wpk = _build_weight_pack(np.asarray(conv_w, dtype=np.float32))
    in_maps = []
    slabs = []
    for core in range(N_CORES):
        n, dc = core // 4, core % 4
        slab = xpad[n, :, 8 * dc : 8 * dc + ZPLANES].reshape(C, XLEN)
        xp2 = np.zeros((128, XBUF), dtype=np.float32)
        xp2[0:64, :XLEN] = slab
        xp2[64:128, : XLEN - 1] = slab[:, 1:]
        in_maps.append({"xp2": xp2, "wpk": wpk})
        slabs.append(np.ascontiguousarray(xp2[0:64]))
    return in_maps, slabs


def kernel(x, conv_w, bn_gamma, bn_beta):
    if "a" not in _PROGRAM_CACHE:
        _PROGRAM_CACHE["a"] = _build_program_a()
        _PROGRAM_CACHE["b"] = _build_program_b()
    nca, ncb = _PROGRAM_CACHE["a"], _PROGRAM_CACHE["b"]

    in_a, slabs = _prep_inputs_a(x, conv_w)
    res_a = bass_utils.run_bass_kernel_spmd(nca, in_a, core_ids=list(range(N_CORES)))

    # host: global BN stats (432 bytes per core), then normalized attention
    st = np.sum([r["st"] for r in res_a.results], axis=0, dtype=np.float64)
    mean = st[:, 0] / M_TOTAL
    var = st[:, 1] / M_TOTAL - mean * mean
    rstd = 1.0 / np.sqrt(var + EPS)
    a = np.asarray(bn_gamma, np.float64) * rstd
    b = np.asarray(bn_beta, np.float64) - mean * a

    in_b = []
    for core in range(N_CORES):
        ssub = res_a.results[core]["ssub"].astype(np.float64)
        e = np.exp(a[:, None] * ssub + b[:, None])
        en = (e / e.sum(axis=0, keepdims=True)).astype(np.float32)
        in_b.append({"xpb": slabs[core], "en": en})
    res_b = bass_utils.run_bass_kernel_spmd(ncb, in_b, core_ids=list(range(N_CORES)))

    full = np.empty((N, C, D // 2, H // 2, W // 2), dtype=np.float32)
    for core in range(N_CORES):
        n, dc = core // 4, core % 4
        full[n, :, 4 * dc : 4 * dc + 4] = res_b.results[core]["out"].reshape(
            64, 4, 16, 16
        )
    return full
